# revision 8
# baseline (speedup 1.0000x reference)
"""MoE gating (nn_MoEGate) Trainium2 Bass kernel.

kernel(x, W_g) -> (vals, idx), matching the jax reference:
    logits = x @ W_g.T ; scores = softmax(logits) ; vals, idx = top_k(scores, 8)

Strategy: data-parallel over the token dim — 8 NeuronCores each process a
4096-token shard with W_g replicated; no cross-core communication.

Per-core pipeline (T=4096 tokens, D=4096, E=64 experts, K=8):
  - x streamed in natural [token, d] layout (contiguous 2MB DMAs).
  - PE transposes 128x128 blocks of x (fp32, exact).
  - ACT copies xT chunks PSUM->SBUF.
  - PE gating matmul, fp32 exact: xT chunk [128d, 128t] stationary, W^T
    chunk [128d, 64e] moving -> logits [128t, 64e] token-major directly,
    accumulated over 32 d-chunks in PSUM.
  - DVE iterative top-8 extraction batched over 1024 tokens, exact
    single-position masking (lax.top_k tie semantics: sorted descending,
    ties -> lowest index).
  - Softmax values: exp on ACT (no max-subtraction needed: |logits| small
    so exp cannot overflow), sum + reciprocal + scale on DVE.
"""
from contextlib import ExitStack

import numpy as np

import concourse.bacc as bacc
import concourse.mybir as mybir
import concourse.tile as tile
from concourse._compat import with_exitstack
from concourse.bass_utils import run_bass_kernel_spmd

F32 = mybir.dt.float32
I32 = mybir.dt.int32
AX = mybir.AxisListType
ALU = mybir.AluOpType
EXP = mybir.ActivationFunctionType.Exp

N_CORES = 8
N_TOKENS = 32768
D = 4096
E = 64
K = 8
T = N_TOKENS // N_CORES  # 4096 tokens per core
NC = D // 128            # 32 d-chunks
GROUP_TOKENS = 512
PAIR_TILES = 8           # token-tiles per topk batch (= 2 groups)
IOBASE = 1087.0


@with_exitstack
def _moe_gate_kernel(ctx: ExitStack, tc: tile.TileContext, outs, ins, n_tokens=T):
    nc = tc.nc
    x_d, w_d = ins
    vals_d, idx_d = outs
    NG = n_tokens // GROUP_TOKENS

    consts = ctx.enter_context(tc.tile_pool(name="consts", bufs=1))
    psum = ctx.enter_context(tc.tile_pool(name="psum", bufs=1, space="PSUM"))

    ident128 = consts.tile([128, 128], F32, tag="ident128")
    nc.vector.memset(ident128[:], 1.0)
    nc.gpsimd.affine_select(
        ident128[:], ident128[:], pattern=[[-1, 128]], compare_op=ALU.is_equal,
        fill=0.0, base=0, channel_multiplier=1,
    )
    ident64 = consts.tile([64, 64], F32, tag="ident64")
    nc.vector.memset(ident64[:], 1.0)
    nc.gpsimd.affine_select(
        ident64[:], ident64[:], pattern=[[-1, 64]], compare_op=ALU.is_equal,
        fill=0.0, base=0, channel_multiplier=1,
    )
    iorev = consts.tile([128, E], F32, tag="iorev")
    nc.gpsimd.iota(
        iorev[:], pattern=[[-1, E]], base=int(IOBASE), channel_multiplier=0,
        allow_small_or_imprecise_dtypes=True,
    )
    negbig = consts.tile([128, 1], F32, tag="negbig")
    nc.vector.memset(negbig[:], -30000.0)

    # W_g^T chunks: wt_sb[128 d, c, 64 e]
    wt_sb = consts.tile([128, NC, E], F32, tag="wt_sb")
    with tc.tile_pool(name="wsetup", bufs=1) as wsp:
        w_sb = wsp.tile([64, D], F32, tag="w_sb")
        for q in range(4):
            nc.sync.dma_start(
                w_sb[:, q * (D // 4):(q + 1) * (D // 4)],
                w_d[:, q * (D // 4):(q + 1) * (D // 4)],
            )
        for cq in range(NC // 8):
            wt_ps = psum.tile([128, 512], F32, tag="xt", bufs=4, name="wt_ps")
            for c8 in range(8):
                c = cq * 8 + c8
                nc.tensor.transpose(
                    wt_ps[:, c8 * 64:(c8 + 1) * 64],
                    w_sb[:, c * 128:(c + 1) * 128],
                    ident64[:],
                )
            nc.vector.tensor_copy(wt_sb[:, cq * 8:(cq + 1) * 8, :], wt_ps[:])

    sb = ctx.enter_context(tc.tile_pool(name="main", bufs=1))

    def topk_batch(lp, tile0, G):
        # lp: [128, G, E] logits for token-tiles [tile0, tile0+G)
        lp_flat = lp[:].rearrange("p g e -> p (g e)")
        exp_t = sb.tile([128, G * E], F32, tag="exp" + str(G), bufs=2, name="exp_t")
        nc.scalar.activation(exp_t[:], lp_flat, EXP)
        den = sb.tile([128, G], F32, tag="den" + str(G), bufs=2, name="den")
        nc.vector.reduce_sum(
            den[:], exp_t[:].rearrange("p (g e) -> p g e", g=G), axis=AX.X
        )
        rden = sb.tile([128, G], F32, tag="rden" + str(G), bufs=2, name="rden")
        nc.vector.reciprocal(rden[:], den[:])

        ws = lp[:, :, :]
        mxs = sb.tile([128, G, K], F32, tag="mxs" + str(G), bufs=2, name="mxs")
        rixs = sb.tile([128, G, K], F32, tag="rixs" + str(G), bufs=2, name="rixs")
        eq = sb.tile([128, G, E], F32, tag="eq" + str(G), bufs=2, name="eq")
        im = sb.tile([128, G, E], F32, tag="im" + str(G), bufs=2, name="im")
        ohm = sb.tile([128, G, E], mybir.dt.uint8, tag="ohm" + str(G), bufs=2, name="ohm")
        io_b = iorev[:].unsqueeze(1).broadcast_to([128, G, E])
        nb_flat = negbig[:].broadcast_to([128, G * E])
        ws_flat = lp[:].rearrange("p g e -> p (g e)")
        for k in range(K):
            nc.vector.reduce_max(mxs[:, :, k], ws, axis=AX.X)
            mx_b = mxs[:, :, k].unsqueeze(2).broadcast_to([128, G, E])
            nc.vector.tensor_tensor(eq[:], ws, mx_b, op=ALU.is_ge)
            nc.vector.tensor_tensor(im[:], eq[:], io_b, op=ALU.mult)
            nc.vector.tensor_reduce(rixs[:, :, k], im[:], axis=AX.X, op=ALU.max)
            if k < K - 1:
                rix_b = rixs[:, :, k].unsqueeze(2).broadcast_to([128, G, E])
                nc.vector.tensor_tensor(ohm[:], im[:], rix_b, op=ALU.is_equal)
                nc.vector.copy_predicated(
                    ws_flat, ohm[:].rearrange("p g e -> p (g e)"), nb_flat
                )

        exk = sb.tile([128, G, K], F32, tag="exk" + str(G), bufs=2, name="exk")
        nc.scalar.activation(exk[:], mxs[:], EXP)
        vals = sb.tile([128, G, K], F32, tag="vals" + str(G), bufs=2, name="vals")
        rden_b = rden[:].unsqueeze(2).broadcast_to([128, G, K])
        nc.vector.tensor_tensor(vals[:], exk[:], rden_b, op=ALU.mult)
        idxf = sb.tile([128, G, K], F32, tag="idxf" + str(G), bufs=2, name="idxf")
        nc.vector.tensor_scalar(
            idxf[:], rixs[:], scalar1=-1.0, scalar2=IOBASE,
            op0=ALU.mult, op1=ALU.add,
        )
        idxi = sb.tile([128, G, K], I32, tag="idxi" + str(G), bufs=2, name="idxi")
        nc.vector.tensor_copy(idxi[:], idxf[:])

        vd = vals_d.rearrange("(t p) k -> p t k", p=128)
        nc.sync.dma_start(vd[:, tile0:tile0 + G, :], vals[:])
        xd = idx_d.rearrange("(t p) k -> p t k", p=128)
        nc.sync.dma_start(xd[:, tile0:tile0 + G, :], idxi[:])

    lp = None
    for g in range(NG):
        x_tiles = []
        # Early groups: split loads along d so the first transposes (which
        # only touch d-chunk 0) can start after ~0.5MB instead of 8MB.
        n_splits = 4 if g == 0 else (2 if g == 1 else 1)
        for j in range(4):
            xt = sb.tile([128, D], F32, tag="x", bufs=9, name=f"x_{g}_{j}")
            x_tiles.append(xt)
        dq = D // n_splits
        for q in range(n_splits):
            for j in range(4):
                r0 = g * GROUP_TOKENS + j * 128
                nc.sync.dma_start(
                    x_tiles[j][:, q * dq:(q + 1) * dq],
                    x_d[r0:r0 + 128, q * dq:(q + 1) * dq],
                )

        # logits for this group, token-major. One PSUM bank (2KB zero
        # region) per accumulation group: [128, 4, 512] = 4 banks, logits
        # in the first 64 cols of each bank.
        lgc_ps = psum.tile([128, 4, 512], F32, tag="lgc", bufs=1, name="lgc_ps")

        def gating(c, xt_sb):
            for j in range(4):
                nc.tensor.matmul(
                    lgc_ps[:, j, 0:E],
                    xt_sb[:, j * 128:(j + 1) * 128],
                    wt_sb[:, c, :],
                    start=(c == 0), stop=(c == NC - 1),
                )

        # software pipeline: gating for chunk c-SKEW is emitted between the
        # transposes of chunk c so the in-order PE stream never waits on the
        # ACT PSUM->SBUF copy of its own chunk.
        SKEW = 2
        xt_sbs = {}
        for c in range(NC):
            xt_ps = psum.tile([128, 512], F32, tag="xt", bufs=4, name="xt_ps")
            for j in range(4):
                nc.tensor.transpose(
                    xt_ps[:, j * 128:(j + 1) * 128],
                    x_tiles[j][:, c * 128:(c + 1) * 128],
                    ident128[:],
                )
            xt_sb = sb.tile([128, 512], F32, tag="xtsb", bufs=6, name="xt_sb")
            nc.scalar.copy(xt_sb[:], xt_ps[:])
            xt_sbs[c] = xt_sb
            if c >= SKEW:
                gating(c - SKEW, xt_sbs.pop(c - SKEW))
        for c in range(NC - SKEW, NC):
            gating(c, xt_sbs.pop(c))

        solo = g >= NG - 2  # last two groups: small topk batches (short tail)
        if solo:
            lps = sb.tile([128, 4, E], F32, tag="lps", bufs=2, name="lps")
            nc.vector.tensor_copy(lps[:, :, :], lgc_ps[:, :, 0:E])
            topk_batch(lps, g * 4, 4)
        else:
            if g % 2 == 0:
                lp = sb.tile([128, PAIR_TILES, E], F32, tag="lp", bufs=3, name="lp")
            half = (g % 2) * 4
            nc.vector.tensor_copy(lp[:, half:half + 4, :], lgc_ps[:, :, 0:E])
            if g % 2 == 1:
                topk_batch(lp, (g - 1) * 4, PAIR_TILES)


_MODEL_CACHE = {}


def build_model(n_tokens=T):
    if n_tokens in _MODEL_CACHE:
        return _MODEL_CACHE[n_tokens]
    nc = bacc.Bacc(
        "TRN2",
        target_bir_lowering=False,
        debug=False,
        enable_asserts=False,
        num_devices=N_CORES,
    )
    x_d = nc.dram_tensor("x", [n_tokens, D], F32, kind="ExternalInput").ap()
    w_d = nc.dram_tensor("w", [E, D], F32, kind="ExternalInput").ap()
    vals_d = nc.dram_tensor("vals", [n_tokens, K], F32, kind="ExternalOutput").ap()
    idx_d = nc.dram_tensor("idx", [n_tokens, K], I32, kind="ExternalOutput").ap()
    with tile.TileContext(nc) as tc:
        _moe_gate_kernel(tc, [vals_d, idx_d], [x_d, w_d], n_tokens=n_tokens)
    nc.compile()
    _MODEL_CACHE[n_tokens] = nc
    return nc


def run_on_cores(x, W_g, trace=False, trace_kwargs=None):
    """x [32768, 4096] f32, W_g [64, 4096] f32 -> (vals, idx), plus results obj."""
    nc = build_model()
    x = np.ascontiguousarray(np.asarray(x, dtype=np.float32))
    W_g = np.ascontiguousarray(np.asarray(W_g, dtype=np.float32))
    shards = np.split(x, N_CORES, axis=0)
    in_maps = [{"x": shards[i], "w": W_g} for i in range(N_CORES)]
    res = run_bass_kernel_spmd(
        nc, in_maps, core_ids=list(range(N_CORES)), trace=trace,
        **(trace_kwargs or {}),
    )
    vals = np.concatenate([r["vals"] for r in res.results], axis=0)
    idx = np.concatenate([r["idx"] for r in res.results], axis=0)
    return (vals, idx), res


def kernel(x, W_g):
    (vals, idx), _ = run_on_cores(x, W_g)
    return vals, idx
